# revision 8
# baseline (speedup 1.0000x reference)
"""Trainium2 Bass kernel for BlockGivensRotation (w @ R, block-diagonal).

The reference applies, per 128-column block of w, 8 sequential sweeps of 127
adjacent-plane Givens rotations.  The composition of all 1016 rotations of a
block is a fixed 128x128 orthogonal matrix R_nb that depends only on `angles`,
so the whole op is `out[:, nb*128:(nb+1)*128] = w[:, nb*128:(nb+1)*128] @ R_nb`
- a block-diagonal matmul, ideal for the tensor engine.

Host side: compose R (tiny: 64x128x128, built in f64 from the 65K angles).
Device side: shard the 64 column-blocks across the 8 cores (8 blocks each) so
every core only needs its own slice of R.  Each core streams w.T tiles from
DRAM, matmuls with the per-block stationary R, and writes out.T tiles back.
w is fed transposed so the contraction dim (block columns) lies on SBUF
partitions with fully contiguous DMA; the host transposes shards in/out.

The kernel is DMA-bound, so all HBM I/O is bf16 (the 2e-2 tolerance leaves
~10x margin over bf16's ~2e-3 rounding): host casts w.T and R to bf16, the
matmul accumulates in f32 PSUM, and the PSUM->SBUF drain casts back to bf16
alternating between the vector and scalar engines so neither becomes the
bottleneck.  That halves HBM traffic vs f32 (32.5 MB/core instead of 64.5),
putting the 8 cores at the chip HBM roofline (~90 us).  DMA queues: w loads
ride the SP ring, R loads and out stores the ACT ring.
"""

import numpy as np
import ml_dtypes

import concourse.bacc as bacc
import concourse.mybir as mybir
import concourse.tile as tile
from concourse.bass_utils import run_bass_kernel_spmd

O = 8192          # w rows
IN_F = 8192       # w cols
B = 128           # Givens block size
NB = IN_F // B    # 64 blocks
NCORES = 8
BPC = NB // NCORES  # 8 column-blocks per core
F32 = mybir.dt.float32
BF16 = mybir.dt.bfloat16
NP_BF16 = ml_dtypes.bfloat16


def _build_rotation_matrices(angles: np.ndarray) -> np.ndarray:
    """Compose the sweeps of adjacent Givens rotations into one 128x128
    matrix per block by applying the reference recurrence to the identity
    (in float64, rounded once at the end)."""
    nb, s, bm1 = angles.shape
    b = bm1 + 1
    ang = np.asarray(angles, dtype=np.float64)
    c = np.cos(ang)
    sn = np.sin(ang)
    R = np.broadcast_to(np.eye(b), (nb, b, b)).copy()  # [NB, basis row, col]
    for sweep in range(s):
        cs, ss = c[:, sweep, :], sn[:, sweep, :]
        carry = R[:, :, 0].copy()
        for i in range(bm1):
            col_j = R[:, :, i + 1]
            ci = cs[:, i][:, None]
            si = ss[:, i][:, None]
            R[:, :, i] = ci * carry - si * col_j
            carry = si * carry + ci * col_j
        R[:, :, b - 1] = carry
    return R


def _build_bass(
    rows=O,
    bpc=BPC,
    ncores=NCORES,
    wt_bufs=3,
    out_bufs=3,
    r_first=1,
    hs=512,
    mm_per_psum=2,
    copy_split=True,
    first_segs=(1024, 3072, 4096),
    mid_segs=(8192,),
    last_segs=(4096, 2048, 1024, 1024),
):
    """Per-core program over this core's `bpc` column-blocks of w:

        out_t[blk*B + c', r] = sum_c R[blk][c, c'] * wt[blk*B + c, r]

    rows: w rows (full, 8192); wt_bufs/out_bufs: pipeline depth;
    r_first: blocks of R in the first (small) R chunk so the first matmul
    isn't gated on the whole R slice; hs: moving free-dim per matmul
    (walrus caps the matmul AP at 512); mm_per_psum: matmuls ganged into
    one PSUM tile so a single wider copy drains them; copy_split:
    alternate the PSUM drain between the vector and scalar engines.
    first/mid/last_segs:
    per-block row tiling - small tiles at the start (fast pipeline ramp),
    one whole-block 2MB contiguous DMA mid-run (fewest instructions, so
    the TileContext semaphore-drain epilogue stays short), small tiles at
    the end (short store tail after the last matmul).
    """
    nc = bacc.Bacc(
        "TRN2", target_bir_lowering=False, debug=False, num_devices=ncores
    )
    wt = nc.dram_tensor("wt", [bpc * B, rows], BF16, kind="ExternalInput")
    r = nc.dram_tensor("r", [B, bpc * B], BF16, kind="ExternalInput")
    out_t = nc.dram_tensor("out_t", [bpc * B, rows], BF16, kind="ExternalOutput")

    with tile.TileContext(nc) as tc:
        with (
            tc.tile_pool(name="rp", bufs=1) as rp,
            tc.tile_pool(name="wtp", bufs=wt_bufs) as wtp,
            tc.tile_pool(name="outp", bufs=out_bufs) as outp,
            tc.tile_pool(
                name="psp", bufs=8 * 512 // (hs * mm_per_psum), space="PSUM"
            ) as psp,
        ):
            # This core's R slice, in two chunks on the ACT ring so it
            # transfers in parallel with the first w tile on SP.
            rf = min(r_first, bpc)
            r_a = rp.tile([B, rf * B], BF16, tag="ra")
            nc.scalar.dma_start(r_a[:], r[:, : rf * B])
            r_b = None
            if rf < bpc:
                r_b = rp.tile([B, (bpc - rf) * B], BF16, tag="rb")
                nc.scalar.dma_start(r_b[:], r[:, rf * B :])
            for blk in range(bpc):
                if blk < rf:
                    r_ap = r_a[:, blk * B : (blk + 1) * B]
                else:
                    r_ap = r_b[:, (blk - rf) * B : (blk - rf + 1) * B]
                plan = (
                    first_segs
                    if blk == 0
                    else (last_segs if blk == bpc - 1 else mid_segs)
                )
                assert sum(plan) == rows
                segs = []
                o = 0
                for seg in plan:
                    segs.append((o, seg))
                    o += seg
                for o, seg in segs:
                    wt_tile = wtp.tile([B, seg], BF16, tag="wt")
                    nc.sync.dma_start(
                        wt_tile[:], wt[blk * B : (blk + 1) * B, o : o + seg]
                    )
                    out_tile = outp.tile([B, seg], BF16, tag="out")
                    # gang mm_per_psum matmuls into one [B, gs] PSUM tile
                    # (2 banks) so one wider copy drains them
                    gs = hs * mm_per_psum
                    ngroups = (seg + gs - 1) // gs
                    for g in range(ngroups):
                        goff = g * gs
                        glen = min(gs, seg - goff)
                        ps = psp.tile([B, glen], F32)
                        for m in range((glen + hs - 1) // hs):
                            mo = m * hs
                            ml = min(hs, glen - mo)
                            nc.tensor.matmul(
                                ps[:, mo : mo + ml],
                                r_ap,
                                wt_tile[:, goff + mo : goff + mo + ml],
                                start=True,
                                stop=True,
                            )
                        dst = out_tile[:, goff : goff + glen]
                        if copy_split and (g % 2 == 1):
                            nc.scalar.copy(dst, ps[:])
                        else:
                            nc.vector.tensor_copy(dst, ps[:])
                    # out-stores ride the second HWDGE ring (ACT)
                    nc.scalar.dma_start(
                        out_t[blk * B : (blk + 1) * B, o : o + seg], out_tile[:]
                    )
    nc.compile()
    return nc


def kernel_impl(w, angles, trace=False, bass_kwargs=None, **spmd_kwargs):
    w = np.asarray(w)
    Rm = _build_rotation_matrices(np.asarray(angles))
    # r_host[c, blk*B + c'] = R[blk][c, c']  (contiguous per SBUF partition c)
    r_host = np.ascontiguousarray(Rm.transpose(1, 0, 2)).reshape(B, NB * B)
    r_host = r_host.astype(NP_BF16)
    nc = _build_bass(**(bass_kwargs or {}))
    csz = BPC * B  # 1024 w-columns per core
    in_maps = [
        {
            "wt": w[:, i * csz : (i + 1) * csz].T.astype(NP_BF16),
            "r": r_host[:, i * csz : (i + 1) * csz],
        }
        for i in range(NCORES)
    ]
    res = run_bass_kernel_spmd(
        nc, in_maps, core_ids=list(range(NCORES)), trace=trace, **spmd_kwargs
    )
    out = np.empty((O, IN_F), dtype=np.float32)
    for i in range(NCORES):
        out[:, i * csz : (i + 1) * csz] = res.results[i]["out_t"].T.astype(
            np.float32
        )
    return out, res


def kernel(w, angles):
    out, _ = kernel_impl(w, angles, trace=False)
    return out
